# revision 5
# baseline (speedup 1.0000x reference)
"""MQA attention kernel (B=2, T=2048, C=2048, 16 query heads, D=128, RoPE,
causal) for 8 Trainium2 NeuronCores.

Sharding: core = (batch, head-group-of-4). Each core computes q projection for
its 4 heads, the full shared K/V projection for its batch (MQA), causal
attention, and a partial output projection; the host sums the 4 partials per
batch.

Device layout notes:
  - Host pre-transposes x to xT [C, T]; contractions over C read contiguous
    DRAM.  Wq/Wk/Wv are host-packed partition-major ([128, 16, m]) and the
    output is packed [128, 16, C] so every DMA descriptor covers >= 4KB per
    partition row (the rings are descriptor-rate bound, not byte bound).
  - x loads as 16 resident full-T k-tiles ([128, 2048] each, 64KB/partition)
    so each projection chunk slices SBUF instead of re-DMAing.
  - RoPE's even/odd interleave is turned into a half-split layout by permuting
    Wq/Wk columns on the host (scores are invariant to a shared permutation of
    the head dim).  Wq is also pre-scaled by 1/sqrt(D).
  - Scores are computed transposed, S.T[j, i], so the p@V and output
    projections need no on-chip transposes; softmax denominators come from an
    all-ones [128,128] stationary matmul over quad-summed exp tiles (3 DVE
    adds buy out 3 PE streams per 4 tiles), inverted with a fast-approx
    reciprocal fused into the PSUM evacuation.
  - Emission is software-pipelined with lag 1: proj(0) proj(1) attn(0)
    proj(2) attn(1) out(0) proj(3) attn(2) out(1) attn(3) out(2) out(3), so
    the PE never waits on RoPE / yt evacuation and out writes spread across
    the whole run.
  - Default dtype is bf16 (~6e-3 rel err); KDT=f32r switches to float32r.
"""

import os
import sys

if "/opt/trn_rl_repo" not in sys.path:
    sys.path.insert(0, "/opt/trn_rl_repo")

import numpy as np

import concourse.bacc as bacc
import concourse.mybir as mybir
import concourse.tile as tile
from concourse.bass_utils import run_bass_kernel_spmd

T = 2048
C = 2048
D = 128
N_HEAD = 16
HPC = 4  # heads per core
N_CORES = 8
F32 = mybir.dt.float32
F32R = mybir.dt.float32r
BF16 = mybir.dt.bfloat16
EXP = mybir.ActivationFunctionType.Exp


KDT = os.environ.get("KDT", "bf16")


def build_program():
    MD = BF16 if KDT == "bf16" else F32R
    AVD = BF16 if KDT in ("bf16", "mix") else F32R
    OD = BF16 if KDT == "bf16" else F32
    nc = bacc.Bacc("TRN2", target_bir_lowering=False, debug=False)

    xt = nc.dram_tensor("xt", [128, 16, T], MD, kind="ExternalInput")
    wq = nc.dram_tensor("wq", [128, 16, HPC * D], MD, kind="ExternalInput")
    wk = nc.dram_tensor("wk", [128, 16, D], MD, kind="ExternalInput")
    wv = nc.dram_tensor("wv", [128, 16, D], MD, kind="ExternalInput")
    wo = nc.dram_tensor("wo", [HPC * D, C], MD, kind="ExternalInput")
    cc = nc.dram_tensor("cc", [D, T], MD, kind="ExternalInput")
    ss = nc.dram_tensor("ss", [D, T], MD, kind="ExternalInput")
    ones_d = nc.dram_tensor("ones_d", [128, 128], AVD, kind="ExternalInput")
    ident_d = nc.dram_tensor("ident_d", [128, 128], MD, kind="ExternalInput")
    out = nc.dram_tensor("out", [128, 16, C], OD, kind="ExternalOutput")

    wo_r = wo.rearrange("(ho p) c -> p ho c", p=128)

    with (
        tile.TileContext(nc) as tc,
        tc.tile_pool(name="consts", bufs=1) as consts,
        tc.tile_pool(name="xpool", bufs=1) as xpool,
        tc.tile_pool(name="qkpool", bufs=20) as qkpool,
        tc.tile_pool(name="ytpool", bufs=16) as ytpool,
        tc.tile_pool(name="wpool", bufs=1) as wpool,
        tc.tile_pool(name="wopool", bufs=1) as wopool,
    ):
        # x: 16 resident full-T k-tiles on the sync queue (4KB descriptors).
        xtk = [
            xpool.tile([128, T], MD, tag=f"xt{k}", name=f"xtk{k}") for k in range(16)
        ]
        for k in range(4):
            nc.sync.dma_start(out=xtk[k], in_=xt[:, k, :])
        # weights interleave on the scalar queue: first quarter of Wq plus
        # K/V so the k<4 accumulation steps can start immediately.
        wqs = wpool.tile([128, 16, 512], MD, tag="w")
        nc.scalar.dma_start(out=wqs[:, 0:4, :], in_=wq[:, 0:4, :])
        wks = consts.tile([128, 16, 128], MD, tag="wk")
        nc.scalar.dma_start(out=wks, in_=wk[:, :, :])
        wvs = consts.tile([128, 16, 128], MD, tag="wv")
        nc.scalar.dma_start(out=wvs, in_=wv[:, :, :])
        for k in range(4, 16):
            nc.sync.dma_start(out=xtk[k], in_=xt[:, k, :])
        for q in range(1, 4):
            nc.scalar.dma_start(out=wqs[:, 4 * q : 4 * q + 4, :], in_=wq[:, 4 * q : 4 * q + 4, :])
        ccs = consts.tile([128, T], MD, tag="cc")
        nc.scalar.dma_start(out=ccs, in_=cc[:, :])
        sss = consts.tile([128, T], MD, tag="ss")
        nc.scalar.dma_start(out=sss, in_=ss[:, :])
        ident = consts.tile([128, 128], MD, tag="ident")
        nc.gpsimd.dma_start(out=ident, in_=ident_d[:, :])
        ones = consts.tile([128, 128], AVD, tag="ones")
        nc.gpsimd.dma_start(out=ones, in_=ones_d[:, :])
        # tri[j, i] = 1 if i >= j else 0 (keep causal-valid entries)
        tri = consts.tile([128, 128], BF16 if KDT in ("bf16", "mix") else F32, tag="tri")
        nc.gpsimd.memset(tri, 1.0)
        nc.gpsimd.affine_select(
            out=tri,
            in_=tri,
            compare_op=mybir.AluOpType.is_ge,
            fill=0.0,
            base=0,
            pattern=[[1, 128]],
            channel_multiplier=-1,
        )
        wos = wopool.tile([128, 4, T], MD, tag="wo")
        for h in range(4):
            nc.gpsimd.dma_start(out=wos[:, h, :], in_=wo_r[:, h, :])
        vsb = [
            consts.tile([128, 128], AVD, tag=f"vsb{j}", name=f"vsb{j}")
            for j in range(16)
        ]  # v, natural [t, d] per j-tile

        # qk[idx][c] = 512-wide chunk c of q.T (idx<4) / k.T (idx=4), RoPE'd
        qk = [
            [qkpool.tile([128, 512], MD, tag="qk", name=f"qk{i}_{c}") for c in range(4)]
            for i in range(5)
        ]
        yt = [
            [ytpool.tile([128, 512], MD, tag="yt", name=f"yt{i}_{c}") for c in range(4)]
            for i in range(4)
        ]

        with (
            tc.tile_pool(name="t512", bufs=8) as t512,
            tc.tile_pool(name="otp", bufs=2) as otp,
            tc.tile_pool(name="psA", bufs=4, space="PSUM") as psA,
            tc.tile_pool(name="psY", bufs=2, space="PSUM") as psY,
            tc.tile_pool(name="psS", bufs=2, space="PSUM") as psS,
        ):
            # ---- q/k/v projection + RoPE for one 512-wide t-chunk ----
            def proj_chunk(tcn):
                tsl = slice(tcn * 512, (tcn + 1) * 512)
                pq = [psA.tile([128, 512], F32, tag="ps", name=f"pq{i}") for i in range(4)]
                pk = psY.tile([128, 512], F32, tag="py", name=f"pk{tcn}")
                pv = psS.tile([128, 512], F32, tag="pss", name=f"pv{tcn}")
                for k in range(16):
                    st, sp = k == 0, k == 15
                    for h in range(4):
                        nc.tensor.matmul(
                            pq[h],
                            wqs[:, k, h * 128 : (h + 1) * 128],
                            xtk[k][:, tsl],
                            start=st,
                            stop=sp,
                        )
                    nc.tensor.matmul(pk, wks[:, k, :], xtk[k][:, tsl], start=st, stop=sp)
                    nc.tensor.matmul(pv, wvs[:, k, :], xtk[k][:, tsl], start=st, stop=sp)
                # v natural tiles for this chunk via PE transpose
                vtt = t512.tile([128, 512], MD, tag="misc", name=f"vtt{tcn}")
                nc.scalar.copy(out=vtt, in_=pv)
                for mm in range(4):
                    m = tcn * 4 + mm
                    ptp = psA.tile([128, 512], MD, tag="ps", name=f"ptp{m}")
                    nc.tensor.transpose(
                        ptp[:, :128], vtt[:, mm * 128 : (mm + 1) * 128], ident
                    )
                    nc.scalar.copy(out=vsb[m], in_=ptp[:, :128])
                for h in range(4):
                    nc.scalar.copy(out=qk[h][tcn], in_=pq[h])
                nc.scalar.copy(out=qk[4][tcn], in_=pk)

                # RoPE this chunk (k first so attention unblocks earliest)
                for idx in [4, 0, 1, 2, 3]:
                    qc = qk[idx][tcn]
                    sw = t512.tile([128, 512], MD, tag="sw", name=f"sw{tcn}_{idx}")
                    nc.gpsimd.dma_start(out=sw[0:64, :], in_=qc[64:128, :])
                    nc.gpsimd.dma_start(out=sw[64:128, :], in_=qc[0:64, :])
                    nc.vector.tensor_mul(out=qc[:], in0=qc[:], in1=ccs[:, tsl])
                    nc.gpsimd.tensor_mul(out=sw[:], in0=sw[:], in1=sss[:, tsl])
                    nc.vector.tensor_add(out=qc[:], in0=qc[:], in1=sw[:])

            # ---- causal attention for one 512-wide i-chunk (transposed
            #      scores S.T[j, i]) ----
            def attn_chunk(c):
                for h in range(4):
                    py = psY.tile([128, 512], F32, tag="py")
                    psm = psS.tile([128, 512], F32, tag="pss")
                    njj = 4 * c + 4
                    pending = []
                    sums_started = False
                    for jj in range(njj):
                        r = jj - 4 * c  # >= 0 only for diagonal-group tiles
                        off = 128 * r if r >= 0 else 0
                        pss = psA.tile([128, 512], F32, tag="ps")
                        nc.tensor.matmul(
                            pss[:, off:],
                            qk[4][jj // 4][:, (jj % 4) * 128 : (jj % 4 + 1) * 128],
                            qk[h][c][:, off:],
                            start=True,
                            stop=True,
                        )
                        pT = t512.tile([128, 512], AVD, tag="pt")
                        nc.scalar.activation(out=pT[:, off:], in_=pss[:, off:], func=EXP)
                        if r >= 0:
                            nc.vector.tensor_mul(
                                out=pT[:, off : off + 128],
                                in0=pT[:, off : off + 128],
                                in1=tri,
                            )
                        nc.tensor.matmul(
                            py[:, off:],
                            vsb[jj],
                            pT[:, off:],
                            start=jj == 0,
                            stop=jj == njj - 1,
                        )
                        # denominator: quad-sum full tiles on DVE (3 adds buy
                        # out 3 of 4 PE streams); diagonal tiles go
                        # individually
                        if r < 0:
                            pending.append(pT)
                            if len(pending) == 4:
                                a = t512.tile([128, 512], AVD, tag="pts")
                                nc.vector.tensor_add(out=a, in0=pending[0], in1=pending[1])
                                b = t512.tile([128, 512], AVD, tag="pts")
                                nc.vector.tensor_add(out=b, in0=pending[2], in1=pending[3])
                                nc.vector.tensor_add(out=a, in0=a, in1=b)
                                nc.tensor.matmul(
                                    psm,
                                    (ones),
                                    a,
                                    start=not sums_started,
                                    stop=False,
                                )
                                sums_started = True
                                pending = []
                        else:
                            nc.tensor.matmul(
                                psm[:, off:],
                                (ones),
                                (pT[:, off:]),
                                start=not sums_started,
                                stop=jj == njj - 1,
                            )
                            sums_started = True
                    bc = t512.tile([128, 512], F32, tag="misc", name=f"bc{c}_{h}")
                    nc.vector.reciprocal_approx_fast(out=bc, in_=psm)
                    nc.vector.tensor_mul(out=yt[h][c], in0=py, in1=bc)

            # ---- partial output projection for one 512-row t-chunk
            #      (contraction over d); one packed 4KB-row store per m ----
            def out_chunk(c):
                for m in range(4 * c, 4 * c + 4):
                    ot = otp.tile([128, C], OD, tag="ot")
                    for cn in range(4):
                        po = psA.tile([128, 512], F32, tag="ps")
                        for h in range(4):
                            nc.tensor.matmul(
                                po,
                                yt[h][m // 4][:, (m % 4) * 128 : (m % 4 + 1) * 128],
                                wos[:, h, cn * 512 : (cn + 1) * 512],
                                start=h == 0,
                                stop=h == 3,
                            )
                        csl = slice(cn * 512, (cn + 1) * 512)
                        if cn % 2 == 0:
                            nc.vector.tensor_copy(out=ot[:, csl], in_=po)
                        else:
                            nc.scalar.copy(out=ot[:, csl], in_=po)
                    nc.sync.dma_start(out=out[:, m, :], in_=ot)

            # software-pipelined emission, lag 1 between dependent stages
            proj_chunk(0)
            proj_chunk(1)
            attn_chunk(0)
            proj_chunk(2)
            attn_chunk(1)
            out_chunk(0)
            proj_chunk(3)
            attn_chunk(2)
            out_chunk(1)
            attn_chunk(3)
            out_chunk(2)
            out_chunk(3)

    nc.compile()
    return nc


_PERM = np.concatenate([np.arange(0, D, 2), np.arange(1, D, 2)])

import ml_dtypes

DT_NP = ml_dtypes.bfloat16 if KDT == "bf16" else np.float32
AV_NP = ml_dtypes.bfloat16 if KDT in ("bf16", "mix") else np.float32


def _pack(a):
    """[C, m] (C = 16*128, row-major) -> [128, 16, m] partition-major."""
    m = a.shape[1]
    return np.ascontiguousarray(a.reshape(16, 128, m).transpose(1, 0, 2))


def make_in_maps(x, freqs_cos, freqs_sin, Wq, Wk, Wv, Wo):
    x = np.asarray(x, dtype=np.float32)
    freqs_cos = np.asarray(freqs_cos, dtype=np.float32)
    freqs_sin = np.asarray(freqs_sin, dtype=np.float32)
    Wq = np.asarray(Wq, dtype=np.float32)
    Wk = np.asarray(Wk, dtype=np.float32)
    Wv = np.asarray(Wv, dtype=np.float32)
    Wo = np.asarray(Wo, dtype=np.float32)

    scale = 1.0 / np.sqrt(np.float32(D))
    cosT = np.ascontiguousarray(freqs_cos.T)  # [64, T]
    sinT = np.ascontiguousarray(freqs_sin.T)
    cc = np.ascontiguousarray(np.concatenate([cosT, cosT], axis=0))  # [128, T]
    ss = np.ascontiguousarray(np.concatenate([-sinT, sinT], axis=0))
    wk_p = _pack(Wk[:, _PERM].astype(DT_NP))
    wv_c = _pack(Wv.astype(DT_NP))

    xts = [_pack(np.ascontiguousarray(x[b].T).astype(DT_NP)) for b in range(2)]

    ones_a = np.ones((128, 128), dtype=AV_NP)
    ident_a = np.eye(128, dtype=DT_NP)
    in_maps = []
    for core in range(N_CORES):
        b = core // 4
        hg = core % 4
        heads = range(4 * hg, 4 * hg + 4)
        qcols = np.concatenate([h * D + _PERM for h in heads])
        wq_c = _pack((Wq[:, qcols] * scale).astype(np.float32).astype(DT_NP))
        orows = np.concatenate([np.arange(h * D, (h + 1) * D) for h in heads])
        wo_c = np.ascontiguousarray(Wo[orows, :])
        in_maps.append(
            {
                "xt": xts[b],
                "wq": wq_c,
                "wk": wk_p,
                "wv": wv_c,
                "wo": wo_c.astype(DT_NP),
                "cc": cc.astype(DT_NP),
                "ss": ss.astype(DT_NP),
                "ones_d": ones_a,
                "ident_d": ident_a,
            }
        )
    return in_maps


_PROGRAM = None


def get_program():
    global _PROGRAM
    if _PROGRAM is None:
        _PROGRAM = build_program()
    return _PROGRAM


def kernel(x, freqs_cos, freqs_sin, Wq, Wk, Wv, Wo, _collect=None):
    nc = get_program()
    in_maps = make_in_maps(x, freqs_cos, freqs_sin, Wq, Wk, Wv, Wo)
    res = run_bass_kernel_spmd(nc, in_maps, core_ids=list(range(N_CORES)))
    if _collect is not None:
        _collect.append(res)
    # out is packed [128, 16, C]; row t = m*128+p lives at [p, m, :]
    outs = [
        np.asarray(r["out"], dtype=np.float32).transpose(1, 0, 2).reshape(T, C)
        for r in res.results
    ]
    full = np.empty((2, T, C), dtype=np.float32)
    for b in range(2):
        full[b] = outs[4 * b] + outs[4 * b + 1] + outs[4 * b + 2] + outs[4 * b + 3]
    return full


# revision 11
# speedup vs baseline: 1.0206x; 1.0206x over previous
"""MQA attention kernel (B=2, T=2048, C=2048, 16 query heads, D=128, RoPE,
causal) for 8 Trainium2 NeuronCores.

Sharding: core = (batch, head-group-of-4). Each core computes q projection for
its 4 heads, the full shared K/V projection for its batch (MQA), causal
attention, and a partial output projection; the host sums the 4 partials per
batch.

Device layout notes:
  - Host pre-transposes x to xT [C, T]; contractions over C read contiguous
    DRAM.  Wq/Wk/Wv are host-packed partition-major ([128, 16, m]) and the
    output is packed [128, 16, C] so every DMA descriptor covers >= 4KB per
    partition row (the rings are descriptor-rate bound, not byte bound).
  - x loads as 16 resident full-T k-tiles ([128, 2048] each, 64KB/partition)
    so each projection chunk slices SBUF instead of re-DMAing.
  - RoPE's even/odd interleave is turned into a half-split layout by permuting
    Wq/Wk columns on the host (scores are invariant to a shared permutation of
    the head dim).  Wq is also pre-scaled by 1/sqrt(D).
  - Scores are computed transposed, S.T[j, i], so the p@V and output
    projections need no on-chip transposes; softmax denominators come from an
    all-ones [128,128] stationary matmul over quad-summed exp tiles (3 DVE
    adds buy out 3 PE streams per 4 tiles), inverted with a fast-approx
    reciprocal fused into the PSUM evacuation.
  - Emission is software-pipelined with lag 1: proj(0) proj(1) attn(0)
    proj(2) attn(1) out(0) proj(3) attn(2) out(1) attn(3) out(2) out(3), so
    the PE never waits on RoPE / yt evacuation and out writes spread across
    the whole run.
  - Default dtype is bf16 (~6e-3 rel err); KDT=f32r switches to float32r.
"""

import os
import sys

if "/opt/trn_rl_repo" not in sys.path:
    sys.path.insert(0, "/opt/trn_rl_repo")

import numpy as np

import concourse.bacc as bacc
import concourse.mybir as mybir
import concourse.tile as tile
from concourse.bass_utils import run_bass_kernel_spmd

T = 2048
C = 2048
D = 128
N_HEAD = 16
HPC = 4  # heads per core
N_CORES = 8
F32 = mybir.dt.float32
F32R = mybir.dt.float32r
BF16 = mybir.dt.bfloat16
EXP = mybir.ActivationFunctionType.Exp


KDT = os.environ.get("KDT", "bf16")


def build_program():
    MD = BF16 if KDT == "bf16" else F32R
    AVD = BF16 if KDT in ("bf16", "mix") else F32R
    OD = BF16 if KDT == "bf16" else F32
    nc = bacc.Bacc("TRN2", target_bir_lowering=False, debug=False)

    xt = nc.dram_tensor("xt", [128, 4, 4, 2048], MD, kind="ExternalInput")
    wq = nc.dram_tensor("wq", [128, 16, HPC * D], MD, kind="ExternalInput")
    wk = nc.dram_tensor("wk", [128, 16, D], MD, kind="ExternalInput")
    wv = nc.dram_tensor("wv", [128, 16, D], MD, kind="ExternalInput")
    wo = nc.dram_tensor("wo", [HPC * D, C], MD, kind="ExternalInput")
    cc = nc.dram_tensor("cc", [D, T], MD, kind="ExternalInput")
    ss = nc.dram_tensor("ss", [D, T], MD, kind="ExternalInput")
    ones_d = nc.dram_tensor("ones_d", [128, 128], AVD, kind="ExternalInput")
    ident_d = nc.dram_tensor("ident_d", [128, 128], MD, kind="ExternalInput")
    out = nc.dram_tensor("out", [128, 16, C], OD, kind="ExternalOutput")

    wo_r = wo.rearrange("(ho p) c -> p ho c", p=128)

    with (
        tile.TileContext(nc) as tc,
        tc.tile_pool(name="consts", bufs=1) as consts,
        tc.tile_pool(name="xpool", bufs=1) as xpool,
        tc.tile_pool(name="qkpool", bufs=20) as qkpool,
        tc.tile_pool(name="ytpool", bufs=16) as ytpool,
        tc.tile_pool(name="wpool", bufs=1) as wpool,
        tc.tile_pool(name="wopool", bufs=1) as wopool,
    ):
        # warmup tile: a few throwaway matmuls burn the PE clock ramp while
        # the first real DMAs land
        dummy = consts.tile([128, 512], MD, tag="dummy")
        nc.gpsimd.memset(dummy, 0.0)
        # x: per-(tcn, group-of-4-k) packed chunks, [128, 4, 512] each with
        # 4KB-per-partition descriptors, paced by a bufs=2 ring per group.
        # weights interleave on the scalar queue: first quarter of Wq plus
        # K/V so the k<4 accumulation steps can start immediately.
        wqs = wpool.tile([128, 16, 512], MD, tag="w")
        nc.scalar.dma_start(out=wqs[:, 0:4, :], in_=wq[:, 0:4, :])
        wks = consts.tile([128, 16, 128], MD, tag="wk")
        nc.scalar.dma_start(out=wks, in_=wk[:, :, :])
        wvs = consts.tile([128, 16, 128], MD, tag="wv")
        nc.scalar.dma_start(out=wvs, in_=wv[:, :, :])
        for q in range(1, 4):
            nc.scalar.dma_start(out=wqs[:, 4 * q : 4 * q + 4, :], in_=wq[:, 4 * q : 4 * q + 4, :])
        ccs = consts.tile([128, T], MD, tag="cc")
        nc.scalar.dma_start(out=ccs, in_=cc[:, :])
        sss = consts.tile([128, T], MD, tag="ss")
        nc.scalar.dma_start(out=sss, in_=ss[:, :])
        ident = consts.tile([128, 128], MD, tag="ident")
        nc.gpsimd.dma_start(out=ident, in_=ident_d[:, :])
        ones = consts.tile([128, 128], AVD, tag="ones")
        nc.gpsimd.dma_start(out=ones, in_=ones_d[:, :])
        # tri[j, i] = 1 if i >= j else 0 (keep causal-valid entries)
        tri = consts.tile([128, 128], BF16 if KDT in ("bf16", "mix") else F32, tag="tri")
        nc.gpsimd.memset(tri, 1.0)
        nc.gpsimd.affine_select(
            out=tri,
            in_=tri,
            compare_op=mybir.AluOpType.is_ge,
            fill=0.0,
            base=0,
            pattern=[[1, 128]],
            channel_multiplier=-1,
        )
        wos = wopool.tile([128, 4, T], MD, tag="wo")
        for h in range(4):
            nc.gpsimd.dma_start(out=wos[:, h, :], in_=wo_r[:, h, :])
        vsb = [
            consts.tile([128, 128], AVD, tag=f"vsb{j}", name=f"vsb{j}")
            for j in range(16)
        ]  # v, natural [t, d] per j-tile

        # qk[idx][c] = 512-wide chunk c of q.T (idx<4) / k.T (idx=4), RoPE'd
        qk = [
            [qkpool.tile([128, 512], MD, tag="qk", name=f"qk{i}_{c}") for c in range(4)]
            for i in range(5)
        ]
        yt = [
            [ytpool.tile([128, 512], MD, tag="yt", name=f"yt{i}_{c}") for c in range(4)]
            for i in range(4)
        ]

        with (
            tc.tile_pool(name="xg", bufs=2) as xg,
            tc.tile_pool(name="t512", bufs=8) as t512,
            tc.tile_pool(name="otp", bufs=2) as otp,
            tc.tile_pool(name="psA", bufs=4, space="PSUM") as psA,
            tc.tile_pool(name="psY", bufs=2, space="PSUM") as psY,
            tc.tile_pool(name="psS", bufs=2, space="PSUM") as psS,
        ):
            # PE clock-ramp warmup during the initial DMA wait (borrows a
            # psA slot; freed before proj(0) needs all four)
            pw = psA.tile([128, 512], F32, tag="ps", name="pw")
            for _ in range(8):
                nc.tensor.matmul(pw, dummy[:, :128], dummy, start=True, stop=True)

            # ---- q/k/v projection for one 512-wide t-chunk ----
            def proj_chunk(tcn):
                tsl = slice(tcn * 512, (tcn + 1) * 512)
                xts = [
                    xg.tile([128, 4, 512], MD, tag=f"xg{g}", name=f"x{tcn}_{g}")
                    for g in range(4)
                ]
                for g in range(4):
                    nc.sync.dma_start(out=xts[g], in_=xt[:, tcn, g, :])
                pq = [psA.tile([128, 512], F32, tag="ps", name=f"pq{i}") for i in range(4)]
                pk = psY.tile([128, 512], F32, tag="py", name=f"pk{tcn}")
                pv = psS.tile([128, 512], F32, tag="pss", name=f"pv{tcn}")
                for k in range(16):
                    xtt = xts[k // 4][:, k % 4, :]
                    st, sp = k == 0, k == 15
                    for h in range(4):
                        nc.tensor.matmul(
                            pq[h],
                            wqs[:, k, h * 128 : (h + 1) * 128],
                            xtt,
                            start=st,
                            stop=sp,
                        )
                    nc.tensor.matmul(pk, wks[:, k, :], xtt, start=st, stop=sp)
                    nc.tensor.matmul(pv, wvs[:, k, :], xtt, start=st, stop=sp)
                # v natural tiles for this chunk via PE transpose
                vtt = t512.tile([128, 512], MD, tag="misc", name=f"vtt{tcn}")
                nc.scalar.copy(out=vtt, in_=pv)
                for mm in range(4):
                    m = tcn * 4 + mm
                    ptp = psA.tile([128, 512], MD, tag="ps", name=f"ptp{m}")
                    nc.tensor.transpose(
                        ptp[:, :128], vtt[:, mm * 128 : (mm + 1) * 128], ident
                    )
                    nc.scalar.copy(out=vsb[m], in_=ptp[:, :128])
                for h in range(4):
                    nc.scalar.copy(out=qk[h][tcn], in_=pq[h])
                nc.scalar.copy(out=qk[4][tcn], in_=pk)

            # ---- RoPE one chunk (k first so attention unblocks earliest);
            #      emitted AFTER attn(tcn-1) so attention's vector ops are
            #      never queued behind RoPE ----
            def rope_chunk(tcn):
                tsl = slice(tcn * 512, (tcn + 1) * 512)
                for idx in [4, 0, 1, 2, 3]:
                    qc = qk[idx][tcn]
                    sw = t512.tile([128, 512], MD, tag="sw", name=f"sw{tcn}_{idx}")
                    nc.gpsimd.dma_start(out=sw[0:64, :], in_=qc[64:128, :])
                    nc.gpsimd.dma_start(out=sw[64:128, :], in_=qc[0:64, :])
                    nc.vector.tensor_mul(out=qc[:], in0=qc[:], in1=ccs[:, tsl])
                    nc.gpsimd.tensor_mul(out=sw[:], in0=sw[:], in1=sss[:, tsl])
                    nc.vector.tensor_add(out=qc[:], in0=qc[:], in1=sw[:])

            # ---- causal attention for one 512-wide i-chunk (transposed
            #      scores S.T[j, i]) ----
            def attn_chunk(c):
                for h in range(4):
                    py = psY.tile([128, 512], F32, tag="py")
                    psm = psS.tile([128, 512], F32, tag="pss")
                    njj = 4 * c + 4
                    pending = []
                    sums_started = False
                    for jj in range(njj):
                        r = jj - 4 * c  # >= 0 only for diagonal-group tiles
                        off = 128 * r if r >= 0 else 0
                        pss = psA.tile([128, 512], F32, tag="ps")
                        nc.tensor.matmul(
                            pss[:, off:],
                            qk[4][jj // 4][:, (jj % 4) * 128 : (jj % 4 + 1) * 128],
                            qk[h][c][:, off:],
                            start=True,
                            stop=True,
                        )
                        pT = t512.tile([128, 512], AVD, tag="pt")
                        nc.scalar.activation(out=pT[:, off:], in_=pss[:, off:], func=EXP)
                        if r >= 0:
                            nc.vector.tensor_mul(
                                out=pT[:, off : off + 128],
                                in0=pT[:, off : off + 128],
                                in1=tri,
                            )
                        nc.tensor.matmul(
                            py[:, off:],
                            vsb[jj],
                            pT[:, off:],
                            start=jj == 0,
                            stop=jj == njj - 1,
                        )
                        # denominator: quad-sum full tiles on DVE (3 adds buy
                        # out 3 of 4 PE streams); diagonal tiles go
                        # individually
                        if r < 0:
                            pending.append(pT)
                            if len(pending) == 4:
                                a = t512.tile([128, 512], AVD, tag="pts")
                                nc.vector.tensor_add(out=a, in0=pending[0], in1=pending[1])
                                b = t512.tile([128, 512], AVD, tag="pts")
                                nc.vector.tensor_add(out=b, in0=pending[2], in1=pending[3])
                                nc.vector.tensor_add(out=a, in0=a, in1=b)
                                nc.tensor.matmul(
                                    psm,
                                    (ones),
                                    a,
                                    start=not sums_started,
                                    stop=False,
                                )
                                sums_started = True
                                pending = []
                        else:
                            nc.tensor.matmul(
                                psm[:, off:],
                                (ones),
                                (pT[:, off:]),
                                start=not sums_started,
                                stop=jj == njj - 1,
                            )
                            sums_started = True
                    bc = t512.tile([128, 512], F32, tag="misc", name=f"bc{c}_{h}")
                    nc.vector.reciprocal_approx_fast(out=bc, in_=psm)
                    nc.vector.tensor_mul(out=yt[h][c], in0=py, in1=bc)

            # ---- partial output projection for one 512-row t-chunk
            #      (contraction over d); one packed 4KB-row store per m ----
            def out_chunk(c):
                for m in range(4 * c, 4 * c + 4):
                    ot = otp.tile([128, C], OD, tag="ot")
                    for cn in range(4):
                        po = psA.tile([128, 512], F32, tag="ps")
                        for h in range(4):
                            nc.tensor.matmul(
                                po,
                                yt[h][m // 4][:, (m % 4) * 128 : (m % 4 + 1) * 128],
                                wos[:, h, cn * 512 : (cn + 1) * 512],
                                start=h == 0,
                                stop=h == 3,
                            )
                        csl = slice(cn * 512, (cn + 1) * 512)
                        if cn % 2 == 0:
                            nc.vector.tensor_copy(out=ot[:, csl], in_=po)
                        else:
                            nc.scalar.copy(out=ot[:, csl], in_=po)
                    nc.sync.dma_start(out=out[:, m, :], in_=ot)

            # software-pipelined emission, lag 1 between dependent stages
            proj_chunk(0)
            rope_chunk(0)
            proj_chunk(1)
            attn_chunk(0)
            rope_chunk(1)
            proj_chunk(2)
            attn_chunk(1)
            rope_chunk(2)
            out_chunk(0)
            proj_chunk(3)
            attn_chunk(2)
            rope_chunk(3)
            out_chunk(1)
            attn_chunk(3)
            out_chunk(2)
            out_chunk(3)

    nc.compile()
    return nc


_PERM = np.concatenate([np.arange(0, D, 2), np.arange(1, D, 2)])

import ml_dtypes

DT_NP = ml_dtypes.bfloat16 if KDT == "bf16" else np.float32
AV_NP = ml_dtypes.bfloat16 if KDT in ("bf16", "mix") else np.float32


def _pack(a):
    """[C, m] (C = 16*128, row-major) -> [128, 16, m] partition-major."""
    m = a.shape[1]
    return np.ascontiguousarray(a.reshape(16, 128, m).transpose(1, 0, 2))


def make_in_maps(x, freqs_cos, freqs_sin, Wq, Wk, Wv, Wo):
    x = np.asarray(x, dtype=np.float32)
    freqs_cos = np.asarray(freqs_cos, dtype=np.float32)
    freqs_sin = np.asarray(freqs_sin, dtype=np.float32)
    Wq = np.asarray(Wq, dtype=np.float32)
    Wk = np.asarray(Wk, dtype=np.float32)
    Wv = np.asarray(Wv, dtype=np.float32)
    Wo = np.asarray(Wo, dtype=np.float32)

    scale = 1.0 / np.sqrt(np.float32(D))
    cosT = np.ascontiguousarray(freqs_cos.T)  # [64, T]
    sinT = np.ascontiguousarray(freqs_sin.T)
    cc = np.ascontiguousarray(np.concatenate([cosT, cosT], axis=0))  # [128, T]
    ss = np.ascontiguousarray(np.concatenate([-sinT, sinT], axis=0))
    wk_p = _pack(Wk[:, _PERM].astype(DT_NP))
    wv_c = _pack(Wv.astype(DT_NP))

    def _pack_x(xb):
        # [C, T] -> [128, tcn(4), g(4), kin*512] with
        # xtp[p, tcn, g, kin*512+t] = xT[(4g+kin)*128+p, tcn*512+t]
        xT = np.ascontiguousarray(xb.T).astype(DT_NP)
        return np.ascontiguousarray(
            xT.reshape(4, 4, 128, 4, 512).transpose(2, 3, 0, 1, 4).reshape(128, 4, 4, 2048)
        )

    xts = [_pack_x(x[b]) for b in range(2)]

    ones_a = np.ones((128, 128), dtype=AV_NP)
    ident_a = np.eye(128, dtype=DT_NP)
    in_maps = []
    for core in range(N_CORES):
        b = core // 4
        hg = core % 4
        heads = range(4 * hg, 4 * hg + 4)
        qcols = np.concatenate([h * D + _PERM for h in heads])
        wq_c = _pack((Wq[:, qcols] * scale).astype(np.float32).astype(DT_NP))
        orows = np.concatenate([np.arange(h * D, (h + 1) * D) for h in heads])
        wo_c = np.ascontiguousarray(Wo[orows, :])
        in_maps.append(
            {
                "xt": xts[b],
                "wq": wq_c,
                "wk": wk_p,
                "wv": wv_c,
                "wo": wo_c.astype(DT_NP),
                "cc": cc.astype(DT_NP),
                "ss": ss.astype(DT_NP),
                "ones_d": ones_a,
                "ident_d": ident_a,
            }
        )
    return in_maps


_PROGRAM = None


def get_program():
    global _PROGRAM
    if _PROGRAM is None:
        _PROGRAM = build_program()
    return _PROGRAM


def kernel(x, freqs_cos, freqs_sin, Wq, Wk, Wv, Wo, _collect=None):
    nc = get_program()
    in_maps = make_in_maps(x, freqs_cos, freqs_sin, Wq, Wk, Wv, Wo)
    res = run_bass_kernel_spmd(nc, in_maps, core_ids=list(range(N_CORES)))
    if _collect is not None:
        _collect.append(res)
    # out is packed [128, 16, C]; row t = m*128+p lives at [p, m, :]
    outs = [
        np.asarray(r["out"], dtype=np.float32).transpose(1, 0, 2).reshape(T, C)
        for r in res.results
    ]
    full = np.empty((2, T, C), dtype=np.float32)
    for b in range(2):
        full[b] = outs[4 * b] + outs[4 * b + 1] + outs[4 * b + 2] + outs[4 * b + 3]
    return full


# revision 16
# speedup vs baseline: 1.1022x; 1.0799x over previous
"""MQA attention kernel (B=2, T=2048, C=2048, 16 query heads, D=128, RoPE,
causal) for 8 Trainium2 NeuronCores.

Sharding: core = (batch, head-group-of-4). Each core computes q projection for
its 4 heads, the full shared K/V projection for its batch (MQA), causal
attention, and a partial output projection; the host sums the 4 partials per
batch.

Device layout notes:
  - Host pre-transposes x to xT [C, T]; contractions over C read contiguous
    DRAM.  Wq/Wk/Wv are host-packed partition-major ([128, 16, m]) and the
    output is packed [128, 16, C] so every DMA descriptor covers >= 4KB per
    partition row (the rings are descriptor-rate bound, not byte bound).
  - x loads as 16 resident full-T k-tiles ([128, 2048] each, 64KB/partition)
    so each projection chunk slices SBUF instead of re-DMAing.
  - RoPE's even/odd interleave is turned into a half-split layout by permuting
    Wq/Wk columns on the host (scores are invariant to a shared permutation of
    the head dim).  Wq is also pre-scaled by 1/sqrt(D).
  - Scores are computed transposed, S.T[j, i], so the p@V and output
    projections need no on-chip transposes; softmax denominators come from an
    all-ones [128,128] stationary matmul over quad-summed exp tiles (3 DVE
    adds buy out 3 PE streams per 4 tiles), inverted with a fast-approx
    reciprocal fused into the PSUM evacuation.
  - Emission is software-pipelined with lag 1: proj(0) proj(1) attn(0)
    proj(2) attn(1) out(0) proj(3) attn(2) out(1) attn(3) out(2) out(3), so
    the PE never waits on RoPE / yt evacuation and out writes spread across
    the whole run.
  - Default dtype is bf16 (~6e-3 rel err); KDT=f32r switches to float32r.
"""

import os
import sys

if "/opt/trn_rl_repo" not in sys.path:
    sys.path.insert(0, "/opt/trn_rl_repo")

import numpy as np

import concourse.bacc as bacc
import concourse.mybir as mybir
import concourse.tile as tile
from concourse.bass_utils import run_bass_kernel_spmd

T = 2048
C = 2048
D = 128
N_HEAD = 16
HPC = 4  # heads per core
N_CORES = 8
F32 = mybir.dt.float32
F32R = mybir.dt.float32r
BF16 = mybir.dt.bfloat16
EXP = mybir.ActivationFunctionType.Exp


KDT = os.environ.get("KDT", "bf16")


def build_program():
    MD = BF16 if KDT == "bf16" else F32R
    AVD = BF16 if KDT in ("bf16", "mix") else F32R
    OD = BF16 if KDT == "bf16" else F32
    nc = bacc.Bacc("TRN2", target_bir_lowering=False, debug=False)

    xt = nc.dram_tensor("xt", [128, 4, 4, 2048], MD, kind="ExternalInput")
    wq = nc.dram_tensor("wq", [128, 16, HPC * D], MD, kind="ExternalInput")
    wk = nc.dram_tensor("wk", [128, 16, D], MD, kind="ExternalInput")
    wv = nc.dram_tensor("wv", [128, 16, D], MD, kind="ExternalInput")
    wo = nc.dram_tensor("wo", [HPC * D, C], MD, kind="ExternalInput")
    cc = nc.dram_tensor("cc", [D, T], MD, kind="ExternalInput")
    ss = nc.dram_tensor("ss", [D, T], MD, kind="ExternalInput")
    ones_d = nc.dram_tensor("ones_d", [128, 128], AVD, kind="ExternalInput")
    ident_d = nc.dram_tensor("ident_d", [128, 128], MD, kind="ExternalInput")
    out = nc.dram_tensor("out", [128, 16, C], OD, kind="ExternalOutput")

    wo_r = wo.rearrange("(ho p) c -> p ho c", p=128)

    with (
        tile.TileContext(nc) as tc,
        tc.tile_pool(name="consts", bufs=1) as consts,
        tc.tile_pool(name="xpool", bufs=1) as xpool,
        tc.tile_pool(name="qkpool", bufs=20) as qkpool,
        tc.tile_pool(name="ytpool", bufs=16) as ytpool,
        tc.tile_pool(name="wpool", bufs=1) as wpool,
        tc.tile_pool(name="wopool", bufs=1) as wopool,
    ):
        # warmup tile: a few throwaway matmuls burn the PE clock ramp while
        # the first real DMAs land
        dummy = consts.tile([128, 512], MD, tag="dummy")
        nc.gpsimd.memset(dummy, 0.0)
        # x: per-(tcn, group-of-4-k) packed chunks, [128, 4, 512] each with
        # 4KB-per-partition descriptors, paced by a bufs=2 ring per group.
        # weights interleave on the scalar queue: first quarter of Wq plus
        # K/V so the k<4 accumulation steps can start immediately.
        wqs = wpool.tile([128, 16, 512], MD, tag="w")
        nc.scalar.dma_start(out=wqs[:, 0:4, :], in_=wq[:, 0:4, :])
        wks = consts.tile([128, 16, 128], MD, tag="wk")
        nc.scalar.dma_start(out=wks, in_=wk[:, :, :])
        wvs = consts.tile([128, 16, 128], MD, tag="wv")
        nc.scalar.dma_start(out=wvs, in_=wv[:, :, :])
        for q in range(1, 4):
            nc.scalar.dma_start(out=wqs[:, 4 * q : 4 * q + 4, :], in_=wq[:, 4 * q : 4 * q + 4, :])
        ccs = consts.tile([128, T], MD, tag="cc")
        nc.scalar.dma_start(out=ccs, in_=cc[:, :])
        sss = consts.tile([128, T], MD, tag="ss")
        nc.scalar.dma_start(out=sss, in_=ss[:, :])
        ident = consts.tile([128, 128], MD, tag="ident")
        nc.scalar.dma_start(out=ident, in_=ident_d[:, :])
        ones = consts.tile([128, 128], AVD, tag="ones")
        nc.scalar.dma_start(out=ones, in_=ones_d[:, :])
        # tri[j, i] = 1 if i >= j else 0 (keep causal-valid entries)
        tri = consts.tile([128, 128], BF16 if KDT in ("bf16", "mix") else F32, tag="tri")
        nc.gpsimd.memset(tri, 1.0)
        nc.gpsimd.affine_select(
            out=tri,
            in_=tri,
            compare_op=mybir.AluOpType.is_ge,
            fill=0.0,
            base=0,
            pattern=[[1, 128]],
            channel_multiplier=-1,
        )
        wos = wopool.tile([128, 4, T], MD, tag="wo")
        vsb = [
            consts.tile([128, 128], AVD, tag=f"vsb{j}", name=f"vsb{j}")
            for j in range(16)
        ]  # v, natural [t, d] per j-tile

        # qk[idx][c] = 512-wide chunk c of q.T (idx<4) / k.T (idx=4), RoPE'd
        qk = [
            [qkpool.tile([128, 512], MD, tag="qk", name=f"qk{i}_{c}") for c in range(4)]
            for i in range(5)
        ]
        yt = [
            [ytpool.tile([128, 512], MD, tag="yt", name=f"yt{i}_{c}") for c in range(4)]
            for i in range(4)
        ]

        with (
            tc.tile_pool(name="xg", bufs=2) as xg,
            tc.tile_pool(name="t512", bufs=8) as t512,
            tc.tile_pool(name="otp", bufs=2) as otp,
            tc.tile_pool(name="psA", bufs=4, space="PSUM") as psA,
            tc.tile_pool(name="psY", bufs=2, space="PSUM") as psY,
            tc.tile_pool(name="psS", bufs=2, space="PSUM") as psS,
        ):
            # PE clock-ramp warmup during the initial DMA wait (borrows a
            # psA slot; freed before proj(0) needs all four)
            pw = psA.tile([128, 512], F32, tag="ps", name="pw")
            for _ in range(8):
                nc.tensor.matmul(pw, dummy[:, :128], dummy, start=True, stop=True)

            # ---- q/k/v projection for one 512-wide t-chunk ----
            def proj_chunk(tcn):
                tsl = slice(tcn * 512, (tcn + 1) * 512)
                xts = [
                    xg.tile([128, 4, 512], MD, tag=f"xg{g}", name=f"x{tcn}_{g}")
                    for g in range(4)
                ]
                for g in range(4):
                    nc.sync.dma_start(out=xts[g], in_=xt[:, tcn, g, :])
                pq = [psA.tile([128, 512], F32, tag="ps", name=f"pq{i}") for i in range(4)]
                pk = psY.tile([128, 512], F32, tag="py", name=f"pk{tcn}")
                pv = psS.tile([128, 512], F32, tag="pss", name=f"pv{tcn}")
                for k in range(16):
                    xtt = xts[k // 4][:, k % 4, :]
                    st, sp = k == 0, k == 15
                    for h in range(4):
                        nc.tensor.matmul(
                            pq[h],
                            wqs[:, k, h * 128 : (h + 1) * 128],
                            xtt,
                            start=st,
                            stop=sp,
                        )
                    nc.tensor.matmul(pk, wks[:, k, :], xtt, start=st, stop=sp)
                    nc.tensor.matmul(pv, wvs[:, k, :], xtt, start=st, stop=sp)
                # v natural tiles for this chunk via PE transpose
                vtt = t512.tile([128, 512], MD, tag="misc", name=f"vtt{tcn}")
                nc.scalar.copy(out=vtt, in_=pv)
                for mm in range(4):
                    m = tcn * 4 + mm
                    ptp = psA.tile([128, 512], MD, tag="ps", name=f"ptp{m}")
                    nc.tensor.transpose(
                        ptp[:, :128], vtt[:, mm * 128 : (mm + 1) * 128], ident
                    )
                    nc.scalar.copy(out=vsb[m], in_=ptp[:, :128])
                for h in range(4):
                    nc.scalar.copy(out=qk[h][tcn], in_=pq[h])
                nc.scalar.copy(out=qk[4][tcn], in_=pk)

            # ---- RoPE one chunk (k first so attention unblocks earliest);
            #      emitted AFTER attn(tcn-1) so attention's vector ops are
            #      never queued behind RoPE ----
            def rope_chunk(tcn):
                tsl = slice(tcn * 512, (tcn + 1) * 512)
                for idx in [4, 0, 1, 2, 3]:
                    qc = qk[idx][tcn]
                    sw = t512.tile([128, 512], MD, tag="sw", name=f"sw{tcn}_{idx}")
                    nc.gpsimd.dma_start(out=sw[0:64, :], in_=qc[64:128, :])
                    nc.gpsimd.dma_start(out=sw[64:128, :], in_=qc[0:64, :])
                    nc.vector.tensor_mul(out=qc[:], in0=qc[:], in1=ccs[:, tsl])
                    nc.gpsimd.tensor_mul(out=sw[:], in0=sw[:], in1=sss[:, tsl])
                    nc.vector.tensor_add(out=qc[:], in0=qc[:], in1=sw[:])
                # stream one quarter of Wo in behind each chunk's RoPE
                nc.gpsimd.dma_start(out=wos[:, tcn, :], in_=wo_r[:, tcn, :])

            # ---- causal attention for one 512-wide i-chunk (transposed
            #      scores S.T[j, i]) ----
            def attn_chunk(c):
                for h in range(4):
                    py = psY.tile([128, 512], F32, tag="py")
                    psm = psS.tile([128, 512], F32, tag="pss")
                    njj = 4 * c + 4
                    pending = []
                    sums_started = False
                    for jj in range(njj):
                        r = jj - 4 * c  # >= 0 only for diagonal-group tiles
                        off = 128 * r if r >= 0 else 0
                        pss = psA.tile([128, 512], F32, tag="ps")
                        nc.tensor.matmul(
                            pss[:, off:],
                            qk[4][jj // 4][:, (jj % 4) * 128 : (jj % 4 + 1) * 128],
                            qk[h][c][:, off:],
                            start=True,
                            stop=True,
                        )
                        pT = t512.tile([128, 512], AVD, tag="pt")
                        nc.scalar.activation(out=pT[:, off:], in_=pss[:, off:], func=EXP)
                        if r >= 0:
                            nc.vector.tensor_mul(
                                out=pT[:, off : off + 128],
                                in0=pT[:, off : off + 128],
                                in1=tri,
                            )
                        nc.tensor.matmul(
                            py[:, off:],
                            vsb[jj],
                            pT[:, off:],
                            start=jj == 0,
                            stop=jj == njj - 1,
                        )
                        # denominator: quad-sum full tiles on DVE (3 adds buy
                        # out 3 of 4 PE streams); diagonal tiles go
                        # individually
                        if r < 0:
                            pending.append(pT)
                            if len(pending) == 4:
                                a = t512.tile([128, 512], AVD, tag="pts")
                                nc.vector.tensor_add(out=a, in0=pending[0], in1=pending[1])
                                b = t512.tile([128, 512], AVD, tag="pts")
                                nc.vector.tensor_add(out=b, in0=pending[2], in1=pending[3])
                                nc.vector.tensor_add(out=a, in0=a, in1=b)
                                nc.tensor.matmul(
                                    psm,
                                    (ones),
                                    a,
                                    start=not sums_started,
                                    stop=False,
                                )
                                sums_started = True
                                pending = []
                        else:
                            nc.tensor.matmul(
                                psm[:, off:],
                                (ones),
                                (pT[:, off:]),
                                start=not sums_started,
                                stop=jj == njj - 1,
                            )
                            sums_started = True
                    bc = t512.tile([128, 512], F32, tag="misc", name=f"bc{c}_{h}")
                    nc.vector.reciprocal_approx_fast(out=bc, in_=psm)
                    nc.vector.tensor_mul(out=yt[h][c], in0=py, in1=bc)

            # ---- partial output projection for one 512-row t-chunk
            #      (contraction over d); one packed 4KB-row store per m ----
            def out_chunk(c):
                for m in range(4 * c, 4 * c + 4):
                    ot = otp.tile([128, C], OD, tag="ot")
                    for cn in range(4):
                        po = psA.tile([128, 512], F32, tag="ps")
                        for h in range(4):
                            nc.tensor.matmul(
                                po,
                                yt[h][m // 4][:, (m % 4) * 128 : (m % 4 + 1) * 128],
                                wos[:, h, cn * 512 : (cn + 1) * 512],
                                start=h == 0,
                                stop=h == 3,
                            )
                        csl = slice(cn * 512, (cn + 1) * 512)
                        nc.scalar.copy(out=ot[:, csl], in_=po)
                    nc.sync.dma_start(out=out[:, m, :], in_=ot)

            # phase-separated emission (interleaving attn between proj
            # segments measurably inflates PE slice times via PSUM
            # contention).  attn starts with c=1 (full tiles first, no
            # tri-mask dependency) and RoPE(3) is emitted after it so
            # attention's vector ops never queue behind RoPE.
            proj_chunk(0)
            rope_chunk(0)
            proj_chunk(1)
            rope_chunk(1)
            proj_chunk(2)
            rope_chunk(2)
            proj_chunk(3)
            attn_chunk(1)
            rope_chunk(3)
            attn_chunk(0)
            out_chunk(1)
            attn_chunk(2)
            out_chunk(0)
            attn_chunk(3)
            out_chunk(2)
            out_chunk(3)

    nc.compile()
    return nc


_PERM = np.concatenate([np.arange(0, D, 2), np.arange(1, D, 2)])

import ml_dtypes

DT_NP = ml_dtypes.bfloat16 if KDT == "bf16" else np.float32
AV_NP = ml_dtypes.bfloat16 if KDT in ("bf16", "mix") else np.float32


def _pack(a):
    """[C, m] (C = 16*128, row-major) -> [128, 16, m] partition-major."""
    m = a.shape[1]
    return np.ascontiguousarray(a.reshape(16, 128, m).transpose(1, 0, 2))


def make_in_maps(x, freqs_cos, freqs_sin, Wq, Wk, Wv, Wo):
    x = np.asarray(x, dtype=np.float32)
    freqs_cos = np.asarray(freqs_cos, dtype=np.float32)
    freqs_sin = np.asarray(freqs_sin, dtype=np.float32)
    Wq = np.asarray(Wq, dtype=np.float32)
    Wk = np.asarray(Wk, dtype=np.float32)
    Wv = np.asarray(Wv, dtype=np.float32)
    Wo = np.asarray(Wo, dtype=np.float32)

    scale = 1.0 / np.sqrt(np.float32(D))
    cosT = np.ascontiguousarray(freqs_cos.T)  # [64, T]
    sinT = np.ascontiguousarray(freqs_sin.T)
    cc = np.ascontiguousarray(np.concatenate([cosT, cosT], axis=0))  # [128, T]
    ss = np.ascontiguousarray(np.concatenate([-sinT, sinT], axis=0))
    wk_p = _pack(Wk[:, _PERM].astype(DT_NP))
    wv_c = _pack(Wv.astype(DT_NP))

    def _pack_x(xb):
        # [C, T] -> [128, tcn(4), g(4), kin*512] with
        # xtp[p, tcn, g, kin*512+t] = xT[(4g+kin)*128+p, tcn*512+t]
        xT = np.ascontiguousarray(xb.T).astype(DT_NP)
        return np.ascontiguousarray(
            xT.reshape(4, 4, 128, 4, 512).transpose(2, 3, 0, 1, 4).reshape(128, 4, 4, 2048)
        )

    xts = [_pack_x(x[b]) for b in range(2)]

    ones_a = np.ones((128, 128), dtype=AV_NP)
    ident_a = np.eye(128, dtype=DT_NP)
    in_maps = []
    for core in range(N_CORES):
        b = core // 4
        hg = core % 4
        heads = range(4 * hg, 4 * hg + 4)
        qcols = np.concatenate([h * D + _PERM for h in heads])
        wq_c = _pack((Wq[:, qcols] * scale).astype(np.float32).astype(DT_NP))
        orows = np.concatenate([np.arange(h * D, (h + 1) * D) for h in heads])
        wo_c = np.ascontiguousarray(Wo[orows, :])
        in_maps.append(
            {
                "xt": xts[b],
                "wq": wq_c,
                "wk": wk_p,
                "wv": wv_c,
                "wo": wo_c.astype(DT_NP),
                "cc": cc.astype(DT_NP),
                "ss": ss.astype(DT_NP),
                "ones_d": ones_a,
                "ident_d": ident_a,
            }
        )
    return in_maps


_PROGRAM = None


def get_program():
    global _PROGRAM
    if _PROGRAM is None:
        _PROGRAM = build_program()
    return _PROGRAM


def kernel(x, freqs_cos, freqs_sin, Wq, Wk, Wv, Wo, _collect=None):
    nc = get_program()
    in_maps = make_in_maps(x, freqs_cos, freqs_sin, Wq, Wk, Wv, Wo)
    res = run_bass_kernel_spmd(nc, in_maps, core_ids=list(range(N_CORES)))
    if _collect is not None:
        _collect.append(res)
    # out is packed [128, 16, C]; row t = m*128+p lives at [p, m, :]
    outs = [
        np.asarray(r["out"], dtype=np.float32).transpose(1, 0, 2).reshape(T, C)
        for r in res.results
    ]
    full = np.empty((2, T, C), dtype=np.float32)
    for b in range(2):
        full[b] = outs[4 * b] + outs[4 * b + 1] + outs[4 * b + 2] + outs[4 * b + 3]
    return full


# revision 20
# speedup vs baseline: 1.1111x; 1.0081x over previous
"""MQA attention kernel (B=2, T=2048, C=2048, 16 query heads, D=128, RoPE,
causal) for 8 Trainium2 NeuronCores.

Sharding: core = (batch, head-group-of-4). Each core computes q projection for
its 4 heads, the full shared K/V projection for its batch (MQA), causal
attention, and a partial output projection; the host sums the 4 partials per
batch.

Device layout notes:
  - Host pre-transposes x to xT [C, T]; contractions over C read contiguous
    DRAM.  Wq/Wk/Wv are host-packed partition-major ([128, 16, m]) and the
    output is packed [128, 16, C] so every DMA descriptor covers >= 4KB per
    partition row (the rings are descriptor-rate bound, not byte bound).
  - x loads as 16 resident full-T k-tiles ([128, 2048] each, 64KB/partition)
    so each projection chunk slices SBUF instead of re-DMAing.
  - RoPE's even/odd interleave is turned into a half-split layout by permuting
    Wq/Wk columns on the host (scores are invariant to a shared permutation of
    the head dim).  Wq is also pre-scaled by 1/sqrt(D).
  - Scores are computed transposed, S.T[j, i], so the p@V and output
    projections need no on-chip transposes; softmax denominators come from an
    all-ones [128,128] stationary matmul over quad-summed exp tiles (3 DVE
    adds buy out 3 PE streams per 4 tiles), inverted with a fast-approx
    reciprocal fused into the PSUM evacuation.
  - Emission is software-pipelined with lag 1: proj(0) proj(1) attn(0)
    proj(2) attn(1) out(0) proj(3) attn(2) out(1) attn(3) out(2) out(3), so
    the PE never waits on RoPE / yt evacuation and out writes spread across
    the whole run.
  - Default dtype is bf16 (~6e-3 rel err); KDT=f32r switches to float32r.
"""

import os
import sys

if "/opt/trn_rl_repo" not in sys.path:
    sys.path.insert(0, "/opt/trn_rl_repo")

import numpy as np

import concourse.bacc as bacc
import concourse.mybir as mybir
import concourse.tile as tile
from concourse.bass_utils import run_bass_kernel_spmd

T = 2048
C = 2048
D = 128
N_HEAD = 16
HPC = 4  # heads per core
N_CORES = 8
F32 = mybir.dt.float32
F32R = mybir.dt.float32r
BF16 = mybir.dt.bfloat16
EXP = mybir.ActivationFunctionType.Exp


KDT = os.environ.get("KDT", "bf16")


def build_program():
    MD = BF16 if KDT == "bf16" else F32R
    AVD = BF16 if KDT in ("bf16", "mix") else F32R
    OD = BF16 if KDT == "bf16" else F32
    nc = bacc.Bacc("TRN2", target_bir_lowering=False, debug=False)

    xt = nc.dram_tensor("xt", [128, 4, 4, 2048], MD, kind="ExternalInput")
    wq = nc.dram_tensor("wq", [128, 16, HPC * D], MD, kind="ExternalInput")
    wk = nc.dram_tensor("wk", [128, 16, D], MD, kind="ExternalInput")
    wv = nc.dram_tensor("wv", [128, 16, D], MD, kind="ExternalInput")
    wo = nc.dram_tensor("wo", [HPC * D, C], MD, kind="ExternalInput")
    cc = nc.dram_tensor("cc", [D, T], MD, kind="ExternalInput")
    ss = nc.dram_tensor("ss", [D, T], MD, kind="ExternalInput")
    ones_d = nc.dram_tensor("ones_d", [128, 128], AVD, kind="ExternalInput")
    ident_d = nc.dram_tensor("ident_d", [128, 128], MD, kind="ExternalInput")
    out = nc.dram_tensor("out", [128, 16, C], OD, kind="ExternalOutput")

    wo_r = wo.rearrange("(ho p) c -> p ho c", p=128)

    with (
        tile.TileContext(nc) as tc,
        tc.tile_pool(name="consts", bufs=1) as consts,
        tc.tile_pool(name="xpool", bufs=1) as xpool,
        tc.tile_pool(name="qkpool", bufs=20) as qkpool,
        tc.tile_pool(name="ytpool", bufs=16) as ytpool,
        tc.tile_pool(name="wpool", bufs=1) as wpool,
        tc.tile_pool(name="wopool", bufs=1) as wopool,
    ):
        # warmup tile: a few throwaway matmuls burn the PE clock ramp while
        # the first real DMAs land
        dummy = consts.tile([128, 512], MD, tag="dummy")
        nc.gpsimd.memset(dummy, 0.0)
        # x: per-(tcn, group-of-4-k) packed chunks, [128, 4, 512] each with
        # 4KB-per-partition descriptors, paced by a bufs=2 ring per group.
        # weights interleave on the scalar queue: first quarter of Wq plus
        # K/V so the k<4 accumulation steps can start immediately.
        wqs = wpool.tile([128, 16, 512], MD, tag="w")
        nc.scalar.dma_start(out=wqs[:, 0:4, :], in_=wq[:, 0:4, :])
        wks = consts.tile([128, 16, 128], MD, tag="wk")
        nc.scalar.dma_start(out=wks, in_=wk[:, :, :])
        wvs = consts.tile([128, 16, 128], MD, tag="wv")
        nc.scalar.dma_start(out=wvs, in_=wv[:, :, :])
        for q in range(1, 4):
            nc.scalar.dma_start(out=wqs[:, 4 * q : 4 * q + 4, :], in_=wq[:, 4 * q : 4 * q + 4, :])
        ccs = consts.tile([128, T], MD, tag="cc")
        nc.scalar.dma_start(out=ccs, in_=cc[:, :])
        sss = consts.tile([128, T], MD, tag="ss")
        nc.scalar.dma_start(out=sss, in_=ss[:, :])
        ident = consts.tile([128, 128], MD, tag="ident")
        nc.scalar.dma_start(out=ident, in_=ident_d[:, :])
        ones = consts.tile([128, 128], AVD, tag="ones")
        nc.scalar.dma_start(out=ones, in_=ones_d[:, :])
        # tri[j, i] = 1 if i >= j else 0 (keep causal-valid entries)
        tri = consts.tile([128, 128], BF16 if KDT in ("bf16", "mix") else F32, tag="tri")
        nc.gpsimd.memset(tri, 1.0)
        nc.gpsimd.affine_select(
            out=tri,
            in_=tri,
            compare_op=mybir.AluOpType.is_ge,
            fill=0.0,
            base=0,
            pattern=[[1, 128]],
            channel_multiplier=-1,
        )
        wos = wopool.tile([128, 4, T], MD, tag="wo")
        vsb = [
            consts.tile([128, 128], AVD, tag=f"vsb{j}", name=f"vsb{j}")
            for j in range(16)
        ]  # v, natural [t, d] per j-tile

        # qk[idx][c] = 512-wide chunk c of q.T (idx<4) / k.T (idx=4), RoPE'd
        qk = [
            [qkpool.tile([128, 512], MD, tag="qk", name=f"qk{i}_{c}") for c in range(4)]
            for i in range(5)
        ]
        yt = [
            [ytpool.tile([128, 512], MD, tag="yt", name=f"yt{i}_{c}") for c in range(4)]
            for i in range(4)
        ]

        with (
            tc.tile_pool(name="xg", bufs=2) as xg,
            tc.tile_pool(name="t512", bufs=8) as t512,
            tc.tile_pool(name="otp", bufs=2) as otp,
            tc.tile_pool(name="psA", bufs=4, space="PSUM") as psA,
            tc.tile_pool(name="psY", bufs=2, space="PSUM") as psY,
            tc.tile_pool(name="psS", bufs=2, space="PSUM") as psS,
        ):
            # PE clock-ramp warmup during the initial DMA wait (borrows a
            # psA slot; freed before proj(0) needs all four)
            pw = psA.tile([128, 512], F32, tag="ps", name="pw")
            for _ in range(8):
                nc.tensor.matmul(pw, dummy[:, :128], dummy, start=True, stop=True)

            # ---- q/k/v projection for one 512-wide t-chunk ----
            def proj_chunk(tcn):
                tsl = slice(tcn * 512, (tcn + 1) * 512)
                xts = [
                    xg.tile([128, 4, 512], MD, tag=f"xg{g}", name=f"x{tcn}_{g}")
                    for g in range(4)
                ]
                for g in range(4):
                    nc.sync.dma_start(out=xts[g], in_=xt[:, tcn, g, :])
                pq = [psA.tile([128, 512], F32, tag="ps", name=f"pq{i}") for i in range(4)]
                pk = psY.tile([128, 512], F32, tag="py", name=f"pk{tcn}")
                pv = psS.tile([128, 512], F32, tag="pss", name=f"pv{tcn}")
                for k in range(16):
                    xtt = xts[k // 4][:, k % 4, :]
                    st, sp = k == 0, k == 15
                    for h in range(4):
                        nc.tensor.matmul(
                            pq[h],
                            wqs[:, k, h * 128 : (h + 1) * 128],
                            xtt,
                            start=st,
                            stop=sp,
                        )
                    nc.tensor.matmul(pk, wks[:, k, :], xtt, start=st, stop=sp)
                    nc.tensor.matmul(pv, wvs[:, k, :], xtt, start=st, stop=sp)
                # v natural tiles for this chunk via PE transpose
                vtt = t512.tile([128, 512], MD, tag="misc", name=f"vtt{tcn}")
                nc.scalar.copy(out=vtt, in_=pv)
                for mm in range(4):
                    m = tcn * 4 + mm
                    ptp = psA.tile([128, 512], MD, tag="ps", name=f"ptp{m}")
                    nc.tensor.transpose(
                        ptp[:, :128], vtt[:, mm * 128 : (mm + 1) * 128], ident
                    )
                    nc.scalar.copy(out=vsb[m], in_=ptp[:, :128])
                for h in range(4):
                    nc.scalar.copy(out=qk[h][tcn], in_=pq[h])
                nc.scalar.copy(out=qk[4][tcn], in_=pk)

            # ---- RoPE one chunk (k first so attention unblocks earliest);
            #      emitted AFTER attn(tcn-1) so attention's vector ops are
            #      never queued behind RoPE ----
            def rope_chunk(tcn):
                tsl = slice(tcn * 512, (tcn + 1) * 512)
                for idx in [4, 0, 1, 2, 3]:
                    qc = qk[idx][tcn]
                    sw = t512.tile([128, 512], MD, tag="sw", name=f"sw{tcn}_{idx}")
                    nc.gpsimd.dma_start(out=sw[0:64, :], in_=qc[64:128, :])
                    nc.gpsimd.dma_start(out=sw[64:128, :], in_=qc[0:64, :])
                    nc.vector.tensor_mul(out=qc[:], in0=qc[:], in1=ccs[:, tsl])
                    nc.gpsimd.tensor_mul(out=sw[:], in0=sw[:], in1=sss[:, tsl])
                    nc.vector.tensor_add(out=qc[:], in0=qc[:], in1=sw[:])
                # stream one quarter of Wo in behind each chunk's RoPE
                nc.gpsimd.dma_start(out=wos[:, tcn, :], in_=wo_r[:, tcn, :])

            # ---- causal attention for one 512-wide i-chunk (transposed
            #      scores S.T[j, i]) ----
            def attn_chunk(c):
                for h in range(4):
                    py = psY.tile([128, 512], F32, tag="py")
                    psm = psS.tile([128, 512], F32, tag="pss")
                    njj = 4 * c + 4
                    pending = []
                    sums_started = False
                    for jj in range(njj):
                        r = jj - 4 * c  # >= 0 only for diagonal-group tiles
                        off = 128 * r if r >= 0 else 0
                        pss = psA.tile([128, 512], F32, tag="ps")
                        nc.tensor.matmul(
                            pss[:, off:],
                            qk[4][jj // 4][:, (jj % 4) * 128 : (jj % 4 + 1) * 128],
                            qk[h][c][:, off:],
                            start=True,
                            stop=True,
                        )
                        pT = t512.tile([128, 512], AVD, tag="pt")
                        nc.scalar.activation(out=pT[:, off:], in_=pss[:, off:], func=EXP)
                        if r >= 0:
                            nc.vector.tensor_mul(
                                out=pT[:, off : off + 128],
                                in0=pT[:, off : off + 128],
                                in1=tri,
                            )
                        nc.tensor.matmul(
                            py[:, off:],
                            vsb[jj],
                            pT[:, off:],
                            start=jj == 0,
                            stop=jj == njj - 1,
                        )
                        # denominator: quad-sum full tiles on DVE (3 adds buy
                        # out 3 of 4 PE streams); diagonal tiles go
                        # individually
                        if r < 0:
                            pending.append(pT)
                            if len(pending) == 4:
                                a = t512.tile([128, 512], AVD, tag="pts")
                                nc.vector.tensor_add(out=a, in0=pending[0], in1=pending[1])
                                b = t512.tile([128, 512], AVD, tag="pts")
                                nc.vector.tensor_add(out=b, in0=pending[2], in1=pending[3])
                                nc.vector.tensor_add(out=a, in0=a, in1=b)
                                nc.tensor.matmul(
                                    psm,
                                    (ones),
                                    a,
                                    start=not sums_started,
                                    stop=False,
                                )
                                sums_started = True
                                pending = []
                        else:
                            nc.tensor.matmul(
                                psm[:, off:],
                                (ones),
                                (pT[:, off:]),
                                start=not sums_started,
                                stop=jj == njj - 1,
                            )
                            sums_started = True
                    bc = t512.tile([128, 512], F32, tag="misc", name=f"bc{c}_{h}")
                    nc.vector.reciprocal_approx_fast(out=bc, in_=psm)
                    nc.vector.tensor_mul(out=yt[h][c], in0=py, in1=bc)

            # ---- partial output projection for one 512-row t-chunk
            #      (contraction over d); one packed 4KB-row store per m ----
            def out_chunk(c, eng="scalar"):
                for m in range(4 * c, 4 * c + 4):
                    ot = otp.tile([128, C], OD, tag="ot")
                    for cn in range(4):
                        po = psA.tile([128, 512], F32, tag="ps")
                        for h in range(4):
                            nc.tensor.matmul(
                                po,
                                yt[h][m // 4][:, (m % 4) * 128 : (m % 4 + 1) * 128],
                                wos[:, h, cn * 512 : (cn + 1) * 512],
                                start=h == 0,
                                stop=h == 3,
                            )
                        csl = slice(cn * 512, (cn + 1) * 512)
                        if eng == "scalar":
                            nc.scalar.copy(out=ot[:, csl], in_=po)
                        else:
                            nc.vector.tensor_copy(out=ot[:, csl], in_=po)
                        if cn == 1:
                            nc.sync.dma_start(out=out[:, m, 0:1024], in_=ot[:, 0:1024])
                    nc.sync.dma_start(out=out[:, m, 1024:2048], in_=ot[:, 1024:2048])

            # phase-separated emission (interleaving attn between proj
            # segments measurably inflates PE slice times via PSUM
            # contention).  attn starts with c=1 (full tiles first, no
            # tri-mask dependency) and RoPE(3) is emitted after it so
            # attention's vector ops never queue behind RoPE.
            proj_chunk(0)
            rope_chunk(0)
            proj_chunk(1)
            rope_chunk(1)
            proj_chunk(2)
            rope_chunk(2)
            proj_chunk(3)
            attn_chunk(1)
            rope_chunk(3)
            attn_chunk(0)
            out_chunk(1)
            attn_chunk(2)
            out_chunk(0)
            attn_chunk(3)
            out_chunk(2, eng="vector")
            out_chunk(3, eng="vector")

    nc.compile()
    return nc


_PERM = np.concatenate([np.arange(0, D, 2), np.arange(1, D, 2)])

import ml_dtypes

DT_NP = ml_dtypes.bfloat16 if KDT == "bf16" else np.float32
AV_NP = ml_dtypes.bfloat16 if KDT in ("bf16", "mix") else np.float32


def _pack(a):
    """[C, m] (C = 16*128, row-major) -> [128, 16, m] partition-major."""
    m = a.shape[1]
    return np.ascontiguousarray(a.reshape(16, 128, m).transpose(1, 0, 2))


def make_in_maps(x, freqs_cos, freqs_sin, Wq, Wk, Wv, Wo):
    x = np.asarray(x, dtype=np.float32)
    freqs_cos = np.asarray(freqs_cos, dtype=np.float32)
    freqs_sin = np.asarray(freqs_sin, dtype=np.float32)
    Wq = np.asarray(Wq, dtype=np.float32)
    Wk = np.asarray(Wk, dtype=np.float32)
    Wv = np.asarray(Wv, dtype=np.float32)
    Wo = np.asarray(Wo, dtype=np.float32)

    scale = 1.0 / np.sqrt(np.float32(D))
    cosT = np.ascontiguousarray(freqs_cos.T)  # [64, T]
    sinT = np.ascontiguousarray(freqs_sin.T)
    cc = np.ascontiguousarray(np.concatenate([cosT, cosT], axis=0))  # [128, T]
    ss = np.ascontiguousarray(np.concatenate([-sinT, sinT], axis=0))
    wk_p = _pack(Wk[:, _PERM].astype(DT_NP))
    wv_c = _pack(Wv.astype(DT_NP))

    def _pack_x(xb):
        # [C, T] -> [128, tcn(4), g(4), kin*512] with
        # xtp[p, tcn, g, kin*512+t] = xT[(4g+kin)*128+p, tcn*512+t]
        xT = np.ascontiguousarray(xb.T).astype(DT_NP)
        return np.ascontiguousarray(
            xT.reshape(4, 4, 128, 4, 512).transpose(2, 3, 0, 1, 4).reshape(128, 4, 4, 2048)
        )

    xts = [_pack_x(x[b]) for b in range(2)]

    ones_a = np.ones((128, 128), dtype=AV_NP)
    ident_a = np.eye(128, dtype=DT_NP)
    in_maps = []
    for core in range(N_CORES):
        b = core // 4
        hg = core % 4
        heads = range(4 * hg, 4 * hg + 4)
        qcols = np.concatenate([h * D + _PERM for h in heads])
        wq_c = _pack((Wq[:, qcols] * scale).astype(np.float32).astype(DT_NP))
        orows = np.concatenate([np.arange(h * D, (h + 1) * D) for h in heads])
        wo_c = np.ascontiguousarray(Wo[orows, :])
        in_maps.append(
            {
                "xt": xts[b],
                "wq": wq_c,
                "wk": wk_p,
                "wv": wv_c,
                "wo": wo_c.astype(DT_NP),
                "cc": cc.astype(DT_NP),
                "ss": ss.astype(DT_NP),
                "ones_d": ones_a,
                "ident_d": ident_a,
            }
        )
    return in_maps


_PROGRAM = None


def get_program():
    global _PROGRAM
    if _PROGRAM is None:
        _PROGRAM = build_program()
    return _PROGRAM


def kernel(x, freqs_cos, freqs_sin, Wq, Wk, Wv, Wo, _collect=None):
    nc = get_program()
    in_maps = make_in_maps(x, freqs_cos, freqs_sin, Wq, Wk, Wv, Wo)
    res = run_bass_kernel_spmd(nc, in_maps, core_ids=list(range(N_CORES)))
    if _collect is not None:
        _collect.append(res)
    # out is packed [128, 16, C]; row t = m*128+p lives at [p, m, :]
    outs = [
        np.asarray(r["out"], dtype=np.float32).transpose(1, 0, 2).reshape(T, C)
        for r in res.results
    ]
    full = np.empty((2, T, C), dtype=np.float32)
    for b in range(2):
        full[b] = outs[4 * b] + outs[4 * b + 1] + outs[4 * b + 2] + outs[4 * b + 3]
    return full
